# revision 11
# baseline (speedup 1.0000x reference)
"""Trainium2 Bass kernel for nn_GCNConvNet (4-layer linear GCN + mean-pool + FC + log_softmax).

Algorithm: the reference network is linear end-to-end (no activations), so the
four GCNConv layers, the final linear, and all biases collapse algebraically:

    logits = M P^4 (x @ R0) + sum_j (M P^(3-j) 1) beta_j + fc_b
    out    = log_softmax(logits)

with P = D^-1/2 (A + 2I) D^-1/2, R0 = W0 W1 W2 W3 fc_w  (128x10), and
beta_j = b_j W_{j+1}..W_3 fc_w (10-vectors). So we propagate a 13-column
state t (10 logits columns + 3 "ones-injection" columns that carry the bias
terms through the remaining P powers) through 4 rounds of P, then mean-pool
per graph, apply the tiny corrections, and log_softmax.

Sharding: nodes are range-partitioned over the 8 cores (6250 each, padded to
6272 = 49*128 slots). Edges are partitioned by destination core and grouped by
512-node destination tile. Each round: every core computes y = dinv * t for
its nodes, an AllGather forms the full f32 y table in DRAM, each core gathers
y[src] for its edges via multi-edge *cluster windows* (each indirect DMA
descriptor pulls a W-row contiguous src window covering every edge whose src
slot falls inside it; windows are bucketed into span classes W in {1,3,5,9}
so each 128-window chunk contributes only W one-hot matmul columns; the DMA
converts f32->fp16 in flight), aggregates per destination tile via one-hot
fp16 matmuls on the tensor engine (messages stationary, psum holds agg^T
[13,512] f32), transposes back to node-major with PE transposes, and applies
the epilogue (self loop + normalization) on the vector engine. One-hots are
built with tensor_scalar is_equal (fp16 iota vs per-partition f32 scalar - no
broadcast op). Pooling is a one-hot matmul against graph ids, AllReduced
across cores, and the final log_softmax runs replicated on every core.

Indirect-DMA cost note: each indirect DMA instruction costs ~1.7us on the
GpSimd engine regardless of descriptor payload (994ns fixed SWDGE overhead +
~0.34ns/descriptor + ~340ns issue gap), and the hardware honors only ONE
offset per partition per instruction (multi-column offset APs read
contiguously from offset[p,0] - CoreSim's batched-gather semantics do NOT
exist on HW, and the Ant extended DMA instructions InstDMAGatherAnt /
InstDMAScatterAddAnt fault in this environment). Cluster windows are
therefore the only way to amortize the per-instruction cost; classes (1,3,5,9)
balance the GpSimd instruction chain against the PE/DVE per-column costs.
"""
import os
import sys

if "/opt/trn_rl_repo" not in sys.path:
    sys.path.insert(0, "/opt/trn_rl_repo")

import numpy as np

import concourse.bacc as bacc
import concourse.bass as bass
import concourse.tile as tile
from concourse import mybir
from concourse import bass_utils

# problem constants (hardcoded per contract)
N = 50000
E = 500000
FIN = 128
G = 50
C = 8  # cores
NPC = N // C  # 6250
GRP = 49  # free-dim groups per core (6272 = 49*128)
SLOTS = GRP * 128  # padded slots per core
NT = 512  # destination-tile width (l-space)
NTILES = (SLOTS + NT - 1) // NT  # 13 (last tile is 128 wide)
F = 13  # propagated columns: 10 logits + 3 bias-injection columns
FE = 14  # pooling rhs columns (F + ones column)
CLASSES = (1, 3, 5, 9)  # window span classes (columns per chunk)

LAST_RESULT = {}


def _host_prep(x, edge_index, batch, Ws, bs, fc_w, fc_b):
    """All integer/index preprocessing + tiny weight algebra on host."""
    src = edge_index[0].astype(np.int64)
    dst = edge_index[1].astype(np.int64)
    batch = batch.astype(np.int64)

    # collapsed weights (float64 for accuracy, cast to f32)
    R4 = fc_w.astype(np.float64)
    R3 = Ws[3].astype(np.float64) @ R4
    R2 = Ws[2].astype(np.float64) @ R3
    R1 = Ws[1].astype(np.float64) @ R2
    R0 = (Ws[0].astype(np.float64) @ R1).astype(np.float32)  # [128,10]
    betas = [
        (bs[0].astype(np.float64) @ R1).astype(np.float32),
        (bs[1].astype(np.float64) @ R2).astype(np.float32),
        (bs[2].astype(np.float64) @ R3).astype(np.float32),
        (bs[3].astype(np.float64) @ R4).astype(np.float32),
    ]

    indeg = np.bincount(dst, minlength=N).astype(np.int64)
    deg = indeg.astype(np.float32) + 2.0
    dinv = (1.0 / np.sqrt(deg)).astype(np.float32)

    # Balanced node -> (core, tile) assignment: serpentine-deal nodes in
    # decreasing in-degree order over the 104 (core, tile) bins so that every
    # bin carries a near-equal number of incoming edges.
    caps = np.full((C, NTILES), NT, np.int64)
    caps[:, NTILES - 1] = NPC - NT * (NTILES - 1)  # 106 real nodes in last tile
    order = np.argsort(-indeg, kind="stable")
    bins = [(c, t) for t in range(NTILES) for c in range(C)]
    fill = {b: [] for b in bins}
    nb = len(bins)
    direction = 1
    idx = 0
    for i in range(N):
        for _ in range(nb + 1):
            b = bins[idx]
            idx += direction
            if idx >= nb:
                idx, direction = nb - 1, -1
            elif idx < 0:
                idx, direction = 0, 1
            if len(fill[b]) < caps[b[0], b[1]]:
                fill[b].append(order[i])
                break
    node2core = np.zeros(N, np.int64)
    node2l = np.zeros(N, np.int64)
    percore_nodes = []
    for c in range(C):
        lst = []
        for t in range(NTILES):
            lst.extend(fill[(c, t)])
        arr = np.asarray(lst, np.int64)
        assert arr.shape[0] == NPC
        percore_nodes.append(arr)
        node2core[arr] = c
        node2l[arr] = np.arange(NPC)

    # global DRAM slot of node n: rank-concat of per-core [6272,13] tables,
    # within core p-major: s = (l%128)*49 + l//128
    gslot = (node2core * SLOTS + (node2l % 128) * GRP + node2l // 128).astype(np.int64)

    WMAX = CLASSES[-1]
    TABROWS = C * SLOTS

    # cluster windows per (core, tile): greedy over src-slot-sorted edges; a
    # window [a, a+W) covers every member edge at its slot gap; same-gap
    # duplicates spill to a fresh window. Windows bucket into span classes.
    def make_clusters(slots_sorted, dloc_sorted):
        """Returns list of (start_slot, [(gap, dloc), ...]) clusters. Edges at
        an already-used window offset (duplicate src slots) are deferred to a
        later pass instead of terminating the cluster."""
        out = []
        work = list(zip(slots_sorted.tolist(), dloc_sorted.tolist()))
        while work:
            pending = []
            i = 0
            n = len(work)
            while i < n:
                s0 = work[i][0]
                start = min(s0, TABROWS - WMAX)
                used = set()
                members = []
                j = i
                while j < n and work[j][0] - start <= WMAX - 1:
                    g = int(work[j][0] - start)
                    if g in used:
                        pending.append(work[j])
                    else:
                        used.add(g)
                        members.append((g, int(work[j][1])))
                    j += 1
                out.append((int(start), members))
                i = j
            work = pending
        return out

    cd = node2core[dst]
    # per core, per tile, per class: list of clusters
    clusters = [[[[] for _ in CLASSES] for _ in range(NTILES)] for _ in range(C)]
    for c in range(C):
        m = cd == c
        s_c = src[m]
        ld = node2l[dst[m]]
        tid = ld // NT
        for t in range(NTILES):
            B = np.where(tid == t)[0]
            gs = gslot[s_c[B]]
            dl = (ld[B] - t * NT).astype(np.int64)
            o = np.argsort(gs, kind="stable")
            for cl in make_clusters(gs[o], dl[o]):
                span = max(g for g, _ in cl[1])
                for k, Wc in enumerate(CLASSES):
                    if span <= Wc - 1:
                        clusters[c][t][k].append(cl)
                        break

    # uniform chunk grid across cores: per (tile, class) chunks = max over cores
    nchunks = np.zeros((NTILES, len(CLASSES)), np.int64)
    for t in range(NTILES):
        for k in range(len(CLASSES)):
            mx = max(len(clusters[c][t][k]) for c in range(C))
            nchunks[t, k] = (mx + 127) // 128

    # chunk/column layout: per tile, classes in order; col base advances by W
    chunk_meta = []  # (tile, classW, chunk_idx_in_class, colbase, srccol)
    colbase = 0
    srccol = 0
    tile_cols = []  # per tile: list of (colbase, W)
    for t in range(NTILES):
        tc = []
        for k, Wc in enumerate(CLASSES):
            for q in range(int(nchunks[t, k])):
                chunk_meta.append((t, Wc, q, colbase, srccol))
                tc.append((colbase, Wc))
                colbase += Wc
                srccol += 1
        tile_cols.append(tc)
    NCOLS = colbase
    NSRC = srccol

    in_maps = []
    for c in range(C):
        srcS = np.zeros((128, NSRC), np.int32)
        dstv = np.full((128, NCOLS), -1.0, np.float32)
        for (t, Wc, q, cb, sc) in chunk_meta:
            k = CLASSES.index(Wc)
            cls = clusters[c][t][k][q * 128 : (q + 1) * 128]
            for p, (start, members) in enumerate(cls):
                srcS[p, sc] = start
                for g, dl in members:
                    dstv[p, cb + g] = np.float32(dl)

        # node-indexed per-core tensors in (p, g) layout (l = 128*g + p)
        def to_pg(v, pad):
            a = np.full(SLOTS, pad, v.dtype)
            a[:NPC] = v
            return a.reshape(GRP, 128).T.copy()

        nodes = percore_nodes[c]
        dinv_c = to_pg(dinv[nodes], np.float32(0.0))
        batch_c = to_pg(batch[nodes].astype(np.float32), np.float32(-1.0))
        xc = np.zeros((SLOTS, FIN), np.float32)
        xc[:NPC] = x[nodes] * dinv[nodes][:, None]  # fold dinv: prolog yields y0
        xT = xc.T.copy()

        R0t = np.zeros((FIN, F), np.float32)
        R0t[:, :10] = R0

        dinvP = np.concatenate(
            [dinv_c, dinv_c * dinv_c, 2 * dinv_c * dinv_c, 2 * dinv_c], axis=1
        )  # [128, 4*49]
        twod2_rep = np.repeat(2 * dinv_c * dinv_c, F, axis=1)  # [128, 49*13]
        twod_rep = np.repeat(2 * dinv_c, F, axis=1)
        batch_rep = np.repeat(batch_c, G, axis=1)  # [128, 49*50]
        iota50_rep = np.tile(np.arange(G, dtype=np.float32), (128, GRP))
        iota512 = np.tile(np.arange(NT, dtype=np.float16), (128, 1))
        ident13 = np.zeros((128, F), np.float32)
        ident13[:F, :F] = np.eye(F, dtype=np.float32)
        fcb_eff = np.tile((fc_b.astype(np.float32) + betas[3])[None, :], (G, 1))
        betac = np.concatenate(
            [np.tile(betas[j][None, :], (G, 1)) for j in range(3)], axis=1
        )  # [50, 30]

        in_maps.append(
            {
                "xT": xT,
                "R0t": R0t,
                "srcS": srcS,
                "dstv": dstv,
                "dinvP": dinvP.astype(np.float32),
                "twod2_rep": twod2_rep.astype(np.float32),
                "twod_rep": twod_rep.astype(np.float32),
                "batch_rep": batch_rep.astype(np.float32),
                "iota50_rep": iota50_rep.astype(np.float32),
                "iota512": iota512,
                "ident13": ident13,
                "fcb_eff": fcb_eff.astype(np.float32),
                "betac": betac.astype(np.float32),
            }
        )
    return in_maps, NCOLS, NSRC, chunk_meta, tile_cols


def _build_kernel(NCOLS, NSRC, chunk_meta, tile_cols):
    nc = bacc.Bacc("TRN2", target_bir_lowering=False, debug=False, num_devices=C, num_swdge_queues=2)
    dt = mybir.dt

    def din(name, shape, dtype=dt.float32):
        return nc.dram_tensor(name, shape, dtype, kind="ExternalInput").ap()

    xT = din("xT", [128, SLOTS])
    R0t = din("R0t", [FIN, F])
    srcS = din("srcS", [128, NSRC], dt.int32)
    dstv = din("dstv", [128, NCOLS], dt.float32)
    dinvP = din("dinvP", [128, 4 * GRP])
    twod2_rep = din("twod2_rep", [128, GRP * F])
    twod_rep = din("twod_rep", [128, GRP * F])
    batch_rep = din("batch_rep", [128, GRP * G])
    iota50_rep = din("iota50_rep", [128, GRP * G])
    iota512 = din("iota512", [128, NT], dt.float16)
    ident13 = din("ident13", [128, F])
    fcb_eff = din("fcb_eff", [G, 10])
    betac = din("betac", [G, 30])
    out = nc.dram_tensor("out", [G, 10], dt.float32, kind="ExternalOutput").ap()

    STT = mybir.AluOpType

    with tile.TileContext(nc) as tc:
        with (
            tc.tile_pool(name="const", bufs=1) as cp,
            tc.tile_pool(name="work", bufs=1) as wp,
            tc.tile_pool(name="spool", bufs=8) as sp,
            tc.tile_pool(name="pa", bufs=3, space="PSUM") as pa,
            tc.tile_pool(name="pb", bufs=2, space="PSUM") as pb,
            tc.tile_pool(name="pc", bufs=2, space="PSUM") as pcp,
            tc.tile_pool(name="dram", bufs=2, space="DRAM") as dp,
        ):
            # ---- load constants ----
            xT_sb = cp.tile([128, SLOTS], dt.float32)
            nc.sync.dma_start(out=xT_sb[:], in_=xT[:])
            R0_sb = cp.tile([FIN, F], dt.float32)
            nc.sync.dma_start(out=R0_sb[:], in_=R0t[:])
            dinv_sb = cp.tile([128, 4 * GRP], dt.float32)
            nc.sync.dma_start(out=dinv_sb[:], in_=dinvP[:])

            d_dinv = dinv_sb[:, 0:GRP]
            d_dinv2 = dinv_sb[:, GRP : 2 * GRP]

            ybufs = [wp.tile([128, GRP, F], dt.float32, name=f"ybuf{i}") for i in range(2)]
            y2a = wp.tile([128, GRP, F], dt.float32)
            rhs14 = wp.tile([128, GRP, FE], dt.float32)
            nc.vector.memset(rhs14[:, :, F : F + 1], 1.0)
            batch_sb = cp.tile([128, GRP, G], dt.float32)
            nc.sync.dma_start(out=batch_sb[:], in_=batch_rep[:].rearrange("p (g j) -> p g j", j=G))
            iota50_sb = cp.tile([128, GRP, G], dt.float32)
            nc.sync.dma_start(out=iota50_sb[:], in_=iota50_rep[:].rearrange("p (g j) -> p g j", j=G))
            Bv = wp.tile([128, GRP, G], dt.float32)
            nc.vector.tensor_tensor(out=Bv[:], in0=batch_sb[:], in1=iota50_sb[:], op=STT.is_equal)
            ps_pool = pcp.tile([G, FE], dt.float32, tag="pc2", bufs=1)

            # ---- prolog: t0 = x @ R0 ; y0 = dinv * t0 (dinv folded into x) ----
            src_sb = cp.tile([128, NSRC], dt.int32)
            nc.sync.dma_start(out=src_sb[:], in_=srcS[:])
            dstv_sb = cp.tile([128, NCOLS], dt.float32)
            nc.sync.dma_start(out=dstv_sb[:], in_=dstv[:])
            twod2_sb = cp.tile([128, GRP, F], dt.float32)
            nc.sync.dma_start(out=twod2_sb[:], in_=twod2_rep[:].rearrange("p (g f) -> p g f", f=F))
            twod_sb = cp.tile([128, GRP, F], dt.float32)
            nc.sync.dma_start(out=twod_sb[:], in_=twod_rep[:].rearrange("p (g f) -> p g f", f=F))
            iota512_sb = cp.tile([128, NT], dt.float16)
            nc.sync.dma_start(out=iota512_sb[:], in_=iota512[:])
            ident_sb = cp.tile([128, F], dt.float32)
            nc.sync.dma_start(out=ident_sb[:], in_=ident13[:])
            for g in range(GRP):
                ps = pcp.tile([128, F], dt.float32, tag="pc")
                nc.tensor.matmul(
                    ps[:], lhsT=xT_sb[:, 128 * g : 128 * (g + 1)], rhs=R0_sb[:],
                    start=True, stop=True,
                )
                nc.scalar.copy(out=ybufs[0][:, g, :], in_=ps[:])

            # ---- 4 propagation rounds ----
            for r in range(4):
                ycur = ybufs[r % 2]
                if r >= 1:
                    # inject the bias-carrier column: y[:, :, 10+r-1] += dinv
                    nc.vector.tensor_tensor(
                        out=ycur[:, :, 10 + r - 1 : 11 + r - 1],
                        in0=ycur[:, :, 10 + r - 1 : 11 + r - 1],
                        in1=d_dinv[:, :, None],
                        op=STT.add,
                    )
                bounce = dp.tile([SLOTS, F], dt.float32, name="bounce")
                nc.sync.dma_start(
                    out=bounce[:].rearrange("(p g) f -> p g f", p=128), in_=ycur[:]
                )
                yfull = dp.tile([C * SLOTS, F], dt.float32, addr_space="Shared", name="yfull")
                nc.gpsimd.collective_compute(
                    "AllGather",
                    STT.bypass,
                    replica_groups=[list(range(C))],
                    ins=[bounce.opt()],
                    outs=[yfull.opt()],
                )

                # whole-buffer self-term: y2a = (2*dinv^2) * y (last round: 2*dinv * y)
                nc.vector.tensor_tensor(
                    out=y2a[:], in0=ycur[:], in1=(twod_sb if r == 3 else twod2_sb)[:],
                    op=STT.mult,
                )

                # gather all cluster windows for this round
                msgs = wp.tile([128, NCOLS, F], dt.float16, name="msgs", bufs=2)
                for chi, (t, Wc, q, cb, sc) in enumerate(chunk_meta):
                    h = nc.gpsimd.indirect_dma_start(
                        out=msgs[:, cb : cb + Wc, :].rearrange("p k d -> p (k d)"),
                        out_offset=None,
                        in_=yfull[:],
                        in_offset=bass.IndirectOffsetOnAxis(ap=src_sb[:, sc : sc + 1], axis=0),
                    )
                    if chi % 2 == 1:
                        h.ins.queue = "qPoolDynamic1"

                # aggregate per destination tile
                dX = d_dinv if r == 3 else d_dinv2
                for t in range(NTILES):
                    ntq = 4 if t < NTILES - 1 else (SLOTS - NT * (NTILES - 1)) // 128
                    width = ntq * 128
                    cols = tile_cols[t]
                    ncol_t = sum(w for _, w in cols)
                    ps_agg = pa.tile([F, NT], dt.float32, tag="agg")
                    ci = 0
                    for (cb, Wc) in cols:
                        for g in range(Wc):
                            j = cb + g
                            S = sp.tile([128, NT], dt.float16, name="S")
                            nc.vector.tensor_scalar(
                                out=S[:, :width],
                                in0=iota512_sb[:, :width],
                                scalar1=dstv_sb[:, j : j + 1],
                                scalar2=None,
                                op0=STT.is_equal,
                            )
                            nc.tensor.matmul(
                                ps_agg[:, :width], lhsT=msgs[:, j, :], rhs=S[:, :width],
                                start=(ci == 0), stop=(ci == ncol_t - 1),
                            )
                            ci += 1
                    for q in range(ntq):
                        g = 4 * t + q
                        sb_q = sp.tile([F, 128], dt.float32, name="sbq", bufs=2)
                        nc.scalar.copy(out=sb_q[:], in_=ps_agg[:, 128 * q : 128 * (q + 1)])
                        ps_t = pb.tile([128, F], dt.float32, tag="tr")
                        nc.tensor.transpose(ps_t[:], sb_q[:], ident_sb[:F, :])
                        dest = rhs14[:, g, :F] if r == 3 else ybufs[(r + 1) % 2][:, g, :]
                        nc.vector.scalar_tensor_tensor(
                            out=dest,
                            in0=ps_t[:],
                            scalar=dX[:, g : g + 1],
                            in1=y2a[:, g, :],
                            op0=STT.mult,
                            op1=STT.add,
                        )
                        if r == 3:
                            nc.tensor.matmul(
                                ps_pool[:], lhsT=Bv[:, g, :], rhs=rhs14[:, g, :],
                                start=(g == 0), stop=(g == GRP - 1),
                            )

            # ---- pooling epilogue (matmuls were folded into round 3) ----
            pool_sb = wp.tile([G, FE], dt.float32)
            nc.scalar.copy(out=pool_sb[:], in_=ps_pool[:])

            ar_in = dp.tile([G, FE], dt.float32, name="arin")
            nc.sync.dma_start(out=ar_in[:], in_=pool_sb[:])
            ar_out = dp.tile([G, FE], dt.float32, addr_space="Shared", name="arout")
            nc.gpsimd.collective_compute(
                "AllReduce", STT.add, replica_groups=[list(range(C))],
                ins=[ar_in.opt()], outs=[ar_out.opt()],
            )
            ps_all = wp.tile([G, FE], dt.float32)
            nc.sync.dma_start(out=ps_all[:], in_=ar_out[:])

            # ---- mean, corrections, log_softmax ----
            fcb_sb = cp.tile([G, 10], dt.float32)
            nc.sync.dma_start(out=fcb_sb[:], in_=fcb_eff[:])
            betac_sb = cp.tile([G, 30], dt.float32)
            nc.sync.dma_start(out=betac_sb[:], in_=betac[:])

            cntm = wp.tile([G, 1], dt.float32)
            nc.vector.tensor_scalar(
                out=cntm[:], in0=ps_all[:, F : F + 1], scalar1=1.0, scalar2=None, op0=STT.max
            )
            rec = wp.tile([G, 1], dt.float32)
            nc.vector.reciprocal(rec[:], cntm[:])
            pooled = wp.tile([G, F], dt.float32)
            nc.vector.tensor_scalar(
                out=pooled[:], in0=ps_all[:, :F], scalar1=rec[:, 0:1], scalar2=None, op0=STT.mult
            )
            logits = wp.tile([G, 10], dt.float32)
            nc.vector.tensor_tensor(out=logits[:], in0=pooled[:, :10], in1=fcb_sb[:], op=STT.add)
            for j in range(3):
                corr = wp.tile([G, 10], dt.float32, name="corr")
                nc.vector.tensor_scalar(
                    out=corr[:], in0=betac_sb[:, 10 * j : 10 * (j + 1)],
                    scalar1=pooled[:, 10 + j : 11 + j], scalar2=None, op0=STT.mult,
                )
                nc.vector.tensor_tensor(out=logits[:], in0=logits[:], in1=corr[:], op=STT.add)

            mx = wp.tile([G, 1], dt.float32)
            nc.vector.reduce_max(mx[:], logits[:], axis=mybir.AxisListType.X)
            sh = wp.tile([G, 10], dt.float32)
            nc.vector.tensor_scalar(
                out=sh[:], in0=logits[:], scalar1=mx[:, 0:1], scalar2=None, op0=STT.subtract
            )
            ex = wp.tile([G, 10], dt.float32)
            nc.scalar.activation(ex[:], sh[:], mybir.ActivationFunctionType.Exp)
            sm = wp.tile([G, 1], dt.float32)
            nc.vector.reduce_sum(sm[:], ex[:], axis=mybir.AxisListType.X)
            ls = wp.tile([G, 1], dt.float32)
            nc.scalar.activation(ls[:], sm[:], mybir.ActivationFunctionType.Ln)
            res = wp.tile([G, 10], dt.float32)
            nc.vector.tensor_scalar(
                out=res[:], in0=sh[:], scalar1=ls[:, 0:1], scalar2=None, op0=STT.subtract
            )
            nc.sync.dma_start(out=out[:], in_=res[:])

    nc.finalize()
    return nc


def kernel(x, edge_index, batch, W0, b0, W1, b1, W2, b2, W3, b3, fc_w, fc_b):
    x = np.asarray(x, np.float32)
    edge_index = np.asarray(edge_index)
    batch = np.asarray(batch)
    Ws = [np.asarray(w, np.float32) for w in (W0, W1, W2, W3)]
    bs = [np.asarray(b, np.float32) for b in (b0, b1, b2, b3)]
    fc_w = np.asarray(fc_w, np.float32)
    fc_b = np.asarray(fc_b, np.float32)

    in_maps, NCOLS, NSRC, chunk_meta, tile_cols = _host_prep(
        x, edge_index, batch, Ws, bs, fc_w, fc_b
    )
    nc = _build_kernel(NCOLS, NSRC, chunk_meta, tile_cols)

    trace = os.environ.get("BASS_TRACE", "0") == "1"
    if os.environ.get("BASS_TRACE"):
        _install_ntff_shim()
    res = bass_utils.run_bass_kernel_spmd(
        nc, in_maps, core_ids=list(range(C)), trace=trace
    )
    LAST_RESULT["exec_time_ns"] = res.exec_time_ns
    LAST_RESULT["results"] = res
    return res.results[0]["out"]


def _install_ntff_shim():
    """antenv.axon_hooks is absent on this image; reconstruct it so
    run_bass_kernel_spmd(trace=True) can NTFF-profile via libaxon_pjrt."""
    import types

    if "antenv.axon_hooks" in sys.modules:
        return
    mod = types.ModuleType("antenv.axon_hooks")
    state = {"hook": None}
    mod.set_axon_ntff_profile_hook = lambda h: state.__setitem__("hook", h)
    mod.get_axon_ntff_profile_hook = lambda: state["hook"]
    sys.modules["antenv.axon_hooks"] = mod
    import antenv

    antenv.axon_hooks = mod
    if "/root/.axon_site" not in sys.path:
        sys.path.append("/root/.axon_site")
    try:
        from trn_agent_boot.trn_boot import _ntff_profile_via_ctypes

        mod.set_axon_ntff_profile_hook(_ntff_profile_via_ctypes("/opt/axon/libaxon_pjrt.so"))
    except Exception:
        pass
